# revision 1
# baseline (speedup 1.0000x reference)
"""Trainium2 Bass kernel for a single attention head with input projections.

Per-batch-element (B=8 -> one NeuronCore each):
  k = key @ Wk^T, q = query @ Wq^T, v = value @ Wv^T          [T, H]
  S = q @ k^T / sqrt(E); mask kidx <= qidx+1; P = softmax(S)
  out = P @ v                                                  [T, H]
T=2048, E=1024, H=2048.

All matmuls run as fp8(e4m3) DoubleRow chains (cost model: 0.5 cyc/row, 4x
cheaper than bf16) with residual compensation to recover ~bf16 accuracy:
every operand x is stored as a pair (x8, dx8) with x8=f8(x), dx8=f8(x-x8),
and each product uses three chains  x8@w8 + dx8@w8 + x8@dw8  (the dx@dw term
is ~2^-8 relative and dropped).  Weights are host-scaled by 16 so their
residuals clear e4m3's subnormal floor; the 1/16 is folded into the act
scale (S) and the ones=16 denominator constant (PV).

Softmax runs unnormalized-transposed as in the bf16 baseline: S^T tiles ->
exp -> P^T, denominator via ones-matmul, normalize at the end.  P is stored
as p8 = e4m3(exp(s)*2^-9) plus an e5m2 residual dp8 = e5(bf16(exp*2^-9)-p8);
e5m2's range covers the softmax tail so both chains accumulate into the
same PSUM with no rescaling.
"""

import math
import sys

sys.path.insert(0, "/opt/trn_rl_repo")

import ml_dtypes
import numpy as np

import concourse.bass as bass
import concourse.mybir as mybir
import concourse.tile as tile
from concourse import bass_utils
from concourse.tile import ScopedClock

B, T, E, H = 8, 2048, 1024, 2048
P = 128
EO = E // P          # 8 e-subtiles
EP = EO // 2         # 4 DoubleRow e-pairs
HO = H // P          # 16 h-subtiles
HP = HO // 2         # 8 DoubleRow h-pairs
TKT = T // P         # 16 tk tiles
NBLK = 4             # tq blocks of 512
BLK = T // NBLK      # 512
CHK = 512            # input-column streaming chunk
WSCALE = 16.0
SSCALE = float(E) ** -0.5 / (WSCALE * WSCALE)  # 2^-13
PBIAS = -9.0 * math.log(2.0)                   # P stored at scale 2^-9
F8D = mybir.dt.float8e4
E5D = mybir.dt.float8e5
BF16 = mybir.dt.bfloat16
F32 = mybir.dt.float32
DRM = mybir.MatmulPerfMode.DoubleRow


class _SplitDrainTC(tile.TileContext):
    """This walrus build rejects >1 sync-wait on the kernel-tail SP Drain
    ("Too many sync wait commands").  Spread the waits over preceding nops
    on the same engine instead — sequentially equivalent."""

    def _drain_and_barrier(self, tick_clock, wait_clock):
        nc = self.nc
        nops = [nc.sync.nop(nofuse=True) for _ in range(40)]
        drain_inst = nc.sync.drain()
        wait_clock.add_sem_waits(
            drain_inst.ins, ScopedClock({None: tick_clock.global_clock})
        )
        si = drain_inst.ins.sync_info
        waits = list(si.on_wait or [])
        if len(waits) > 1:
            assert len(waits) <= len(nops) + 1
            si.on_wait = [waits[-1]]
            for w, nop in zip(waits[:-1], nops):
                nsi = nop.ins.sync_info
                if nsi is None:
                    nop.ins.sync_info = mybir.SyncInfo(on_wait=[w], on_update=[])
                else:
                    nsi.on_wait = [w]
        nc.all_engine_barrier()
        popped = nc._tile_sem_poison_stack.pop()
        assert popped is self._sem_poison
        nc.clear_and_free_semaphores(list(self.sems.allocated().values()))
        nc.all_engine_barrier()


def _build():
    nc = bass.Bass("TRN2", target_bir_lowering=False, debug=False)

    dram = {}
    for nm in ("xq", "xk", "xv"):
        for r in ("8", "r"):
            dram[nm + r] = nc.dram_tensor(nm + r, (E, T), F8D,
                                          kind="ExternalInput").ap()
    for nm in ("wq", "wk", "wv"):
        for r in ("8", "r"):
            dram[nm + r] = nc.dram_tensor(nm + r, (E, H), F8D,
                                          kind="ExternalInput").ap()
    masks = nc.dram_tensor("masks", (P, 2 * BLK), F8D, kind="ExternalInput").ap()
    out = nc.dram_tensor("out", (T, H), F32, kind="ExternalOutput").ap()

    def et(a):  # [E, X] dram -> [128, EO, X] view
        return a.rearrange("(eo p) t -> p eo t", p=P)

    def chain(ps_ap, ops):
        n = len(ops)
        for i, (lhsT, rhs, pm) in enumerate(ops):
            nc.tensor.matmul(ps_ap, lhsT, rhs, start=(i == 0),
                             stop=(i == n - 1), perf_mode=pm)

    class PsRot:
        """Rotate psum allocation over several pools: deeper effective ring
        than any one pool, so WAR sem-waits land far behind the PE head
        (micro-gaps reset the p-state ramp and cost ~1.5us each)."""

        def __init__(self, slots):
            self.slots = slots
            self.i = 0

        def tile(self, shape):
            pool, tag = self.slots[self.i % len(self.slots)]
            self.i += 1
            return pool.tile(shape, F32, tag=tag, name=f"psr_{tag}_{self.i}")

    with _SplitDrainTC(nc) as tc:
        with (
            tc.tile_pool(name="w", bufs=1) as w_pool,
            tc.tile_pool(name="x", bufs=2) as x_pool,
            tc.tile_pool(name="kt", bufs=1) as kt_pool,
            tc.tile_pool(name="vv", bufs=1) as v_pool,
            tc.tile_pool(name="qt", bufs=1) as q_pool,
            tc.tile_pool(name="pt", bufs=1) as p_pool,
            tc.tile_pool(name="pbf", bufs=2) as pbf_pool,
            tc.tile_pool(name="outs", bufs=2) as out_pool,
            tc.tile_pool(name="misc", bufs=1) as misc_pool,
            tc.tile_pool(name="dkst", bufs=4) as dkst_pool,
            tc.tile_pool(name="dktr", bufs=4) as dktr_pool,
            tc.tile_pool(name="ps_p", bufs=2, space="PSUM") as ps_proj,
            tc.tile_pool(name="ps_s", bufs=3, space="PSUM") as ps_s,
            tc.tile_pool(name="ps_o", bufs=2, space="PSUM") as ps_o,
            tc.tile_pool(name="ps_d", bufs=1, space="PSUM") as ps_d,
            tc.tile_pool(name="dram", bufs=1, space="DRAM") as dram_pool,
        ):
            rotA = PsRot([(ps_proj, "pp"), (ps_s, "s512"), (ps_o, "po")])
            rotQ = PsRot([(ps_proj, "pp"), (ps_s, "s512")])
            rotS = PsRot([(ps_s, "s512")])
            rotPV = PsRot([(ps_o, "po")])

            masks_sb = misc_pool.tile([P, 2 * BLK], F8D, tag="masks")
            nc.sync.dma_start(masks_sb[:], masks)
            ones8 = misc_pool.tile([P, 2, 1], F8D, tag="ones8")
            nc.vector.memset(ones8[:], WSCALE)
            ones5 = misc_pool.tile([P, 2, 1], E5D, tag="ones5")
            nc.vector.memset(ones5[:], WSCALE)
            bias_sb = misc_pool.tile([P, 1], F32, tag="bias")
            nc.vector.memset(bias_sb[:], PBIAS)

            # PE warm-up: ~5us of zero matmuls with no DMA dependencies keep
            # the p-state ramp hot while the first weight/input DMAs land
            # (post-idle instructions run at half clock for 3us otherwise)
            wuA = misc_pool.tile([P, 2, P], F8D, tag="wuA")
            nc.vector.memset(wuA[:], 0.0)
            wuB = misc_pool.tile([P, 2, BLK], F8D, tag="wuB")
            nc.vector.memset(wuB[:], 0.0)
            wups = rotA.tile([P, BLK])
            chain(wups[:], [(wuA[:], wuB[:], DRM) for _ in range(28)])

            kt8 = kt_pool.tile([P, HO, T], F8D, tag="kt8")
            # k-residual doesn't fit in SBUF alongside everything else: it
            # round-trips through DRAM and is streamed per k-tile in phase B
            dkt_dram = dram_pool.tile([P, HO, T], F8D)
            v8 = v_pool.tile([P, TKT, H], F8D, tag="v8")
            dv8 = v_pool.tile([P, TKT, H], F8D, tag="dv8")

            def load_w(nm, after=None):
                """Load a weight pair as per-512-column-chunk tiles (finer
                WAR granularity lets the next phase's load start before this
                phase fully drains); `after` is a callback run after the
                first chunk's DMAs so the first compute chain's other inputs
                aren't queued behind the full 4 MiB weight load."""
                w8, dw8 = [], []
                got = None
                for c in range(NBLK):
                    cs = slice(c * BLK, (c + 1) * BLK)
                    wc = w_pool.tile([P, EO, BLK], F8D, tag=f"w8c{c}")
                    dwc = w_pool.tile([P, EO, BLK], F8D, tag=f"dw8c{c}")
                    nc.sync.dma_start(wc[:], et(dram[nm + "8"])[:, :, cs])
                    nc.sync.dma_start(dwc[:], et(dram[nm + "r"])[:, :, cs])
                    w8.append(wc)
                    dw8.append(dwc)
                    if c == 0 and after is not None:
                        got = after()

                class WView:
                    """Column-sliceable view over the 4 chunk tiles; only
                    slices within one chunk are ever requested."""

                    def __init__(self, chunks):
                        self.chunks = chunks

                    def col(self, lo, hi):  # absolute H columns -> chunk AP
                        c = lo // BLK
                        assert hi <= (c + 1) * BLK
                        return self.chunks[c], lo - c * BLK, hi - c * BLK

                return WView(w8), WView(dw8), got

            def load_x(nm, off):
                x8 = x_pool.tile([P, EO, CHK], F8D, tag="x8")
                dx8 = x_pool.tile([P, EO, CHK], F8D, tag="dx8")
                hf = CHK // 2
                for t, d in ((x8, dram[nm + "8"]), (dx8, dram[nm + "r"])):
                    # two DMAs per tensor for queue parallelism
                    nc.sync.dma_start(t[:, :, :hf], et(d)[:, :, off : off + hf])
                    nc.sync.dma_start(t[:, :, hf:],
                                      et(d)[:, :, off + hf : off + CHK])
                return x8, dx8

            # ---- Phase A1: kT (scaled) resident as fp8 pair [128, HO, T]
            w8, dw8, x0 = load_w("wk", after=lambda: load_x("xk", 0))
            for cb in range(T // CHK):
                x8, dx8 = x0 if cb == 0 else load_x("xk", cb * CHK)
                for ho in range(HO):
                    wc, lo, hi = w8.col(ho * P, (ho + 1) * P)
                    dwc, _, _ = dw8.col(ho * P, (ho + 1) * P)
                    ps = rotA.tile([P, CHK])
                    if cb == 0 and ho == 0:
                        # quarter-chains: start the PE as soon as the first
                        # 128 input columns land instead of the whole chunk
                        for q0 in range(0, CHK, P):
                            qs = slice(q0, q0 + P)
                            ops = []
                            for L, R in ((wc, x8), (dwc, x8), (wc, dx8)):
                                ops += [(L[:, 2 * e : 2 * e + 2, lo:hi],
                                         R[:, 2 * e : 2 * e + 2, qs], DRM)
                                        for e in range(EP)]
                            chain(ps[:, qs], ops)
                    else:
                        ops = []
                        for L, R in ((wc, x8), (dwc, x8), (wc, dx8)):
                            ops += [(L[:, 2 * e : 2 * e + 2, lo:hi],
                                     R[:, 2 * e : 2 * e + 2, :], DRM)
                                    for e in range(EP)]
                        chain(ps[:], ops)
                    cs = slice(cb * CHK, (cb + 1) * CHK)
                    nc.scalar.copy(kt8[:, ho, cs], ps[:])
                    dkst = dkst_pool.tile([P, CHK], F8D, tag="dkst")
                    nc.vector.tensor_tensor(
                        dkst[:], ps[:], kt8[:, ho, cs],
                        mybir.AluOpType.subtract,
                    )
                    nc.sync.dma_start(dkt_dram[:, ho, cs], dkst[:])

            # ---- Phase A2: v (scaled) resident as fp8 pair [128, TKT, H]
            w8, dw8, x0 = load_w("wv", after=lambda: load_x("xv", 0))
            for tp in range(T // CHK):
                x8, dx8 = x0 if tp == 0 else load_x("xv", tp * CHK)
                for tl in range(CHK // P):
                    tt = (CHK // P) * tp + tl
                    ts = slice(tl * P, (tl + 1) * P)
                    for hb in range(NBLK):
                        hs = slice(hb * BLK, (hb + 1) * BLK)
                        wc, _, _ = w8.col(hb * BLK, (hb + 1) * BLK)
                        dwc, _, _ = dw8.col(hb * BLK, (hb + 1) * BLK)
                        ps = rotA.tile([P, BLK])
                        ops = []
                        for L, R in ((x8, wc), (dx8, wc), (x8, dwc)):
                            ops += [(L[:, 2 * e : 2 * e + 2, ts],
                                     R[:, 2 * e : 2 * e + 2, :], DRM)
                                    for e in range(EP)]
                        chain(ps[:], ops)
                        nc.scalar.copy(v8[:, tt, hs], ps[:])
                        nc.vector.tensor_tensor(
                            dv8[:, tt, hs], ps[:], v8[:, tt, hs],
                            mybir.AluOpType.subtract,
                        )

            # ---- Phase B: per tq block of 512
            w8, dw8, x0 = load_w("wq", after=lambda: load_x("xq", 0))
            q8 = q_pool.tile([P, HO, BLK], F8D, tag="q8")
            dq8 = q_pool.tile([P, HO, BLK], F8D, tag="dq8")
            p8 = p_pool.tile([P, TKT, BLK], F8D, tag="p8")
            dp8 = p_pool.tile([P, TKT, BLK], E5D, tag="dp8")
            # third P component for the j=0 block: early rows average few
            # keys, so the subnormal-zone p8+dp8 error doesn't wash out
            ddp8 = p_pool.tile([P, 5, BLK], E5D, tag="ddp8")
            def qproj_chains(j):
                """qT block [128, HO, 512] fp8 pair; yields after each ho
                chain so it can interleave with the previous block's PV."""
                for cb in range(BLK // CHK):
                    if j == 0 and cb == 0:
                        x8, dx8 = x0
                    else:
                        x8, dx8 = load_x("xq", j * BLK + cb * CHK)
                    cs = slice(cb * CHK, (cb + 1) * CHK)
                    for ho in range(HO):
                        wc, lo, hi = w8.col(ho * P, (ho + 1) * P)
                        dwc, _, _ = dw8.col(ho * P, (ho + 1) * P)
                        ps = rotQ.tile([P, CHK])
                        ops = []
                        for L, R in ((wc, x8), (dwc, x8), (wc, dx8)):
                            ops += [(L[:, 2 * e : 2 * e + 2, lo:hi],
                                     R[:, 2 * e : 2 * e + 2, :], DRM)
                                    for e in range(EP)]
                        chain(ps[:], ops)
                        nc.scalar.copy(q8[:, ho, cs], ps[:])
                        nc.vector.tensor_tensor(
                            dq8[:, ho, cs], ps[:], q8[:, ho, cs],
                            mybir.AluOpType.subtract,
                        )
                        yield

            for _ in qproj_chains(0):
                pass
            for j in range(NBLK):
                ntk = min(4 * j + 5, TKT)

                # S^T tiles -> exp -> (p8, dp8), mask-trimmed on the diagonal
                def load_dkt(t):
                    r = dktr_pool.tile([P, HO, P], F8D, tag="dktr",
                                       name=f"dktr_{j}_{t}")
                    nc.sync.dma_start(r[:], dkt_dram[:, :, t * P : (t + 1) * P])
                    return r

                ring = {t: load_dkt(t) for t in range(min(3, ntk))}
                for t in range(ntk):
                    if t + 3 < ntk:
                        ring[t + 3] = load_dkt(t + 3)
                    m = t - 4 * j
                    # the +1 superdiagonal means tile 4j+m is visible from
                    # column 128m-1 on; zero the unwritten head of the
                    # s-block that straddles it so den/PV chains read zeros
                    start = 0 if m < 1 else P * m - 1
                    if m >= 1:
                        zs = slice(P * (m - 1), start)
                        comps = (p8, dp8, ddp8) if j == 0 else (p8, dp8)
                        for pt in comps:
                            nc.vector.memset(pt[:, t, zs], 0.0)
                    width = BLK - start
                    tks = slice(t * P, (t + 1) * P)
                    ps = rotS.tile([P, BLK])
                    ops = []
                    for h in range(HP):
                        hp = slice(2 * h, 2 * h + 2)
                        ops.append((kt8[:, hp, tks], q8[:, hp, start:], DRM))
                        ops.append((ring[t][:, hp, :], q8[:, hp, start:], DRM))
                        ops.append((kt8[:, hp, tks], dq8[:, hp, start:], DRM))
                    chain(ps[:, start:], ops)
                    nc.scalar.activation(
                        p8[:, t, start:], ps[:, start:],
                        mybir.ActivationFunctionType.Exp,
                        scale=SSCALE, bias=bias_sb[:],
                    )
                    pbf = pbf_pool.tile([P, BLK], BF16, tag="pbf")
                    nc.scalar.activation(
                        pbf[:, start:], ps[:, start:],
                        mybir.ActivationFunctionType.Exp,
                        scale=SSCALE, bias=bias_sb[:],
                    )
                    if j == 0:
                        dpbf = pbf_pool.tile([P, BLK], BF16, tag="pbf")
                        nc.vector.tensor_tensor(
                            dpbf[:, start:], pbf[:, start:], p8[:, t, start:],
                            mybir.AluOpType.subtract,
                        )
                        nc.vector.tensor_copy(dp8[:, t, start:], dpbf[:, start:])
                        nc.vector.tensor_tensor(
                            ddp8[:, t, start:], dpbf[:, start:],
                            dp8[:, t, start:], mybir.AluOpType.subtract,
                        )
                    elif j != 3:
                        nc.vector.tensor_tensor(
                            dp8[:, t, start:], pbf[:, start:], p8[:, t, start:],
                            mybir.AluOpType.subtract,
                        )
                    if m >= 0:
                        moff = BLK - P * m + start  # = 512 (m=0) or 384
                        comps = ((p8, dp8, ddp8) if j == 0 else
                                 (p8,) if j == 3 else (p8, dp8))
                        for pt in comps:
                            nc.vector.tensor_tensor(
                                pt[:, t, start:], pt[:, t, start:],
                                masks_sb[:, moff : moff + width],
                                mybir.AluOpType.mult,
                            )

                # denominator: den[tq] = 16 * sum_tk (p8 + dp8)
                den_ps = ps_d.tile([P, NBLK], F32)
                for s in range(NBLK):
                    ntk_s = min(4 * j + s + 2, TKT)
                    scol = slice(s * P, (s + 1) * P)
                    # j=3 drops the P-residual from num AND den (consistent
                    # truncated softmax): late rows average ~1800 keys, so the
                    # p8-only weight noise stays inside the error budget
                    pchains = [(p8, ones8)] if j == 3 else \
                        [(p8, ones8), (dp8, ones5)]
                    if j == 0:
                        pchains.append((ddp8, ones5))
                    ops = []
                    for pt, ones in pchains:
                        ops += [(pt[:, 2 * u : 2 * u + 2, scol], ones[:], DRM)
                                for u in range(ntk_s // 2)]
                        if ntk_s % 2:
                            ops.append((pt[:, ntk_s - 1, scol], ones[:, 0], None))
                    chain(den_ps[:, s : s + 1], ops)
                recip_sb = misc_pool.tile([P, NBLK], F32, tag=f"recip{j}")
                nc.vector.reciprocal(recip_sb[:], den_ps[:])

                # out[tq, h] = (P^T.T @ v) / den, per (hb, s).  The next
                # block's q-projection interleaves 1:1 with these chains:
                # its pp/s512 psums and ACT/DVE consumers are free here, and
                # the extra in-flight work hides the 2-deep po-ring waits.
                nextq = qproj_chains(j + 1) if j + 1 < NBLK else iter(())
                for hb in range(NBLK):
                    hs = slice(hb * BLK, (hb + 1) * BLK)
                    for s in range(NBLK):
                        ntk_s = min(4 * j + s + 2, TKT)
                        scol = slice(s * P, (s + 1) * P)
                        o_ps = rotPV.tile([P, BLK])
                        pv_chains = [(p8, v8), (p8, dv8)] if j == 3 else \
                            [(p8, v8), (dp8, v8), (p8, dv8)]
                        if j == 0:
                            pv_chains += [(ddp8, v8), (dp8, dv8)]
                        ops = []
                        for Pc, Vc in pv_chains:
                            ops += [(Pc[:, 2 * u : 2 * u + 2, scol],
                                     Vc[:, 2 * u : 2 * u + 2, hs], DRM)
                                    for u in range(ntk_s // 2)]
                            if ntk_s % 2:
                                ops.append((Pc[:, ntk_s - 1, scol],
                                            Vc[:, ntk_s - 1, hs], None))
                        chain(o_ps[:], ops)
                        o_sb = out_pool.tile([P, BLK], F32, tag="o")
                        nc.vector.tensor_scalar_mul(
                            o_sb[:], o_ps[:], recip_sb[:, s : s + 1]
                        )
                        nc.sync.dma_start(
                            out[j * BLK + s * P : j * BLK + (s + 1) * P, hs],
                            o_sb[:],
                        )
                        next(nextq, None)
                for _ in nextq:
                    pass
    return nc


def _split_waits(nc, limit=1):
    """This walrus build accepts only one sync-wait per TPB instruction.
    Move excess waits onto same-engine nops inserted just before the
    instruction (engine sequencers execute in order, so this is
    semantically identical)."""
    k = 0
    for f in nc.m.functions:
        for blk in f.blocks:
            new = []
            for inst in blk.instructions:
                si = inst.sync_info
                waits = list(si.on_wait) if si and si.on_wait else []
                if len(waits) > limit:
                    for w in waits[:-limit]:
                        nop = mybir.InstNoOp(name=f"wsplit-{k}", ins=[], outs=[])
                        k += 1
                        nop.engine = inst.engine
                        nop.sync_info = mybir.SyncInfo(on_wait=[w], on_update=[])
                        new.append(nop)
                    si.on_wait = waits[-limit:]
                new.append(inst)
            blk.instructions[:] = new
    return nc


_NC_CACHE = None


def _get_nc():
    global _NC_CACHE
    if _NC_CACHE is None:
        _NC_CACHE = _split_waits(_build())
    return _NC_CACHE


def _host_masks():
    # wide[p, c] = (p <= c - 511); slices give the partial-tile masks.
    p = np.arange(P)[:, None]
    c = np.arange(2 * BLK)[None, :]
    return (p <= c - (BLK - 1)).astype(ml_dtypes.float8_e4m3)


def _pair(x):
    f8 = ml_dtypes.float8_e4m3
    x8 = x.astype(f8)
    dx8 = (x - x8.astype(np.float32)).astype(f8)
    return x8, dx8


def _prep_in_maps(key, query, value, Wk, Wq, Wv):
    ws = {}
    for nm, W in (("wq", Wq), ("wk", Wk), ("wv", Wv)):
        wt = np.ascontiguousarray(W.T).astype(np.float32) * WSCALE  # [E, H]
        ws[nm + "8"], ws[nm + "r"] = _pair(wt)
    masks = _host_masks()
    in_maps = []
    for b in range(B):
        m = dict(ws, masks=masks)
        for nm, x in (("xq", query), ("xk", key), ("xv", value)):
            xt = np.ascontiguousarray(x[b].T).astype(np.float32)  # [E, T]
            m[nm + "8"], m[nm + "r"] = _pair(xt)
        in_maps.append(m)
    return in_maps


def kernel(key, query, value, Wk, Wq, Wv):
    nc = _get_nc()
    in_maps = _prep_in_maps(key, query, value, Wk, Wq, Wv)
    res = bass_utils.run_bass_kernel_spmd(nc, in_maps, core_ids=list(range(B)))
    return np.stack([res.results[i]["out"] for i in range(B)]).astype(np.float32)

